# revision 14
# baseline (speedup 1.0000x reference)
"""Trainium2 Bass kernel for nn_Cross_Attention (B=16, C=256, H=W=96).

reference:
    q = Z1.reshape(B, C, N); k = Zr.reshape(B, C, N)         # N = H*W
    energy    = q @ k^T                                       # [B, C, C]
    attention = softmax(rowmax(energy) - energy, axis=-1)
    out       = attention @ k                                 # [B, C, N]
    return beta * out + Zr

Strategy: data-parallel over batch, 2 batches per NeuronCore on 8 cores.
All device I/O is fp8e4m3 and all matmuls run in fp8 with DoubleRow perf
mode (one PE instruction contracts a 256-deep pair of k-tiles), which cuts
both the HBM traffic and the PE time vs a bf16 formulation:
  - q^T is host-packed fp8 [P, 36, 2, C] (contraction-pair-major) so the
    energy matmul streams straight from DRAM with no on-chip transposes.
  - k  is the fp8 downcast of Zr, loaded once [C, N]; the energy matmul's
    k^T pair-tiles are derived on-chip with PE transpose-mode matmuls.
    fp8 transposes must write PSUM with element step 2 and 4-byte-aligned
    starts (walrus rules), so the psum tiles carry a pad byte per element;
    the psum->SBUF repack copies the padded region as packed uint16 - the
    VectorE 2x_1p mode then moves it at half cost - and the energy matmul
    reads the SBUF tile through a stride-2 fp8 view.
  - softmax(max - e) == exp(min - e) / sum(exp(min - e)) row-wise: only a
    row-min is needed, exp args are <= 0 (no overflow), sum >= 1.
  - beta and 1/sum are folded into the attention weights BEFORE the second
    matmul, so the device emits delta := beta * (attn @ k) in fp8 and the
    host adds the f32 residual:  out = Zr + delta.  When beta == 0 the
    folded weights are exactly zero, delta is exactly zero, and the
    returned output is bitwise Zr.
The kernel is a two-batch software pipeline ordered so every in-order
engine queue sees its work in data-arrival order; psum->SBUF repacks are
balanced across ScalarE/VectorE; mm2 psum chunks rotate through the outp
banks plus (once transposes retire) the borrowed transpose banks.
"""

from contextlib import ExitStack

import ml_dtypes
import numpy as np

import concourse.bass as bass
import concourse.tile as tile
from concourse import bacc, mybir
from concourse.bass_utils import run_bass_kernel_spmd
from concourse.masks import make_identity

B, C, H, W = 16, 256, 96, 96
N = H * W                    # 9216
P = 128
NCORES = 8
BL = B // NCORES             # 2 batches per core
CT = C // P                  # 2 c-tiles of 128
NT = N // P                  # 72 contraction tiles
NPAIR = NT // 2              # 36 DoubleRow contraction pairs
QCH = 12                     # qt pairs per DMA chunk -> 3 chunks
NQC = NPAIR // QCH           # 3 qt chunks
GP = 2                       # pairs per transpose/repack group
NG = NPAIR // GP             # 18 groups per batch
KCC = 4                      # kb column chunks per c-tile row
KCW = N // KCC               # 2304 cols per kb chunk
OW = 512                     # mm2 psum chunk width
DCW = 2 * OW                 # 1024: outp-bank delta repack width
NDC = N // DCW               # 9 delta repacks per c-tile row
SCW = 3 * DCW                # 3072: batch-0 store width
NIL = 12                     # batch-0 mm2 chunks interleaved into phase B

F32 = mybir.dt.float32
U16 = mybir.dt.uint16
FP8 = mybir.dt.float8e4
NP_FP8 = ml_dtypes.float8_e4m3
DR = mybir.MatmulPerfMode.DoubleRow


class _CopyBalancer:
    """Route psum->SBUF repack copies to ScalarE or VectorE by projected
    busy-ns (GpSimd cannot access PSUM).  Only the DVE has the 2x_1p fast
    path for 2-byte packed operands; `prefer` pins a stream to one engine
    unless it is further ahead than `slack` ns."""

    def __init__(self, nc):
        self.engines = [
            [nc.scalar.copy, 0.81, 0.81, 210.0, 0.0],
            [lambda out, in_: nc.vector.tensor_copy(out=out, in_=in_),
             0.95, 0.43, 218.0, 0.0],
        ]

    def charge(self, idx, ns):
        self.engines[idx][4] += ns

    def copy(self, out, in_, free, twox=False, prefer=None, slack=2000.0):
        r = 2 if twox else 1
        best = min(self.engines, key=lambda e: e[4] + free * e[r] + e[3])
        if prefer is not None:
            p = self.engines[prefer]
            if p[4] + free * p[r] + p[3] <= best[4] + free * best[r] + best[3] + slack:
                best = p
        best[4] += free * best[r] + best[3]
        if best is self.engines[0]:
            best[0](out=out, in_=in_)
        else:
            best[0](out, in_)


def _build_program():
    nc = bacc.Bacc("TRN2", target_bir_lowering=False, debug=False,
                   num_devices=NCORES)

    qt_ext = nc.dram_tensor("qt", [BL, P, NPAIR, 2, C], FP8,
                            kind="ExternalInput")
    zr_ext = nc.dram_tensor("zr", [BL, C, N], FP8, kind="ExternalInput")
    beta_ext = nc.dram_tensor("beta", [1], F32, kind="ExternalInput")
    out_ext = nc.dram_tensor("out", [BL, C, N], FP8, kind="ExternalOutput")

    with tile.TileContext(nc) as tc, ExitStack() as ctx:
        kbp = ctx.enter_context(tc.tile_pool(name="kbp", bufs=2))
        qtp = ctx.enter_context(tc.tile_pool(name="qtp", bufs=6))
        kttp = ctx.enter_context(tc.tile_pool(name="kttp", bufs=38))
        expp = ctx.enter_context(tc.tile_pool(name="expp", bufs=2))
        attp = ctx.enter_context(tc.tile_pool(name="attp", bufs=2))
        atTp = ctx.enter_context(tc.tile_pool(name="atTp", bufs=4))
        deltap = ctx.enter_context(tc.tile_pool(name="deltap", bufs=4))
        statp = ctx.enter_context(tc.tile_pool(name="statp", bufs=8))
        singles = ctx.enter_context(tc.tile_pool(name="singles", bufs=1))
        engp = ctx.enter_context(tc.tile_pool(name="engp", bufs=1, space="PSUM"))
        trp = ctx.enter_context(tc.tile_pool(name="trp", bufs=3, space="PSUM"))
        outp = ctx.enter_context(tc.tile_pool(name="outp", bufs=2, space="PSUM"))

        cb = _CopyBalancer(nc)

        # ---- all loads up front on the sync queue.  DMA device order =
        # [kb(b0), qt(b0), kb(b1)/qt(b1) interleaved, stores...]: the last
        # arrival for each batch feeds the shortest dependency chain ----
        kb = [kbp.tile([P, CT, N], FP8, name="kb") for _ in range(BL)]
        qt = [[qtp.tile([P, QCH, 2, C], FP8, name="qt_t")
               for _ in range(NQC)] for _ in range(BL)]

        def load_kb(b, cc):
            for cj in range(CT):
                nc.sync.dma_start(
                    out=kb[b][:, cj, cc * KCW:(cc + 1) * KCW],
                    in_=zr_ext[b, cj * P:(cj + 1) * P,
                               cc * KCW:(cc + 1) * KCW])

        def load_qt(b, cc):
            nc.sync.dma_start(
                out=qt[b][cc], in_=qt_ext[b, :, cc * QCH:(cc + 1) * QCH, :, :])

        for cc in range(KCC):
            load_kb(0, cc)
        for cc in range(NQC):
            load_qt(0, cc)
        # batch 1: kb and qt interleaved; the final piece is qt (its energy
        # matmuls are the cheapest consumers, so the b1 tail starts sooner)
        load_kb(1, 0)
        load_qt(1, 0)
        load_kb(1, 1)
        load_qt(1, 1)
        load_kb(1, 2)
        load_kb(1, 3)
        load_qt(1, 2)

        ident = singles.tile([P, P], FP8)
        make_identity(nc, ident)
        beta_sb = singles.tile([P, 1], F32)
        nc.gpsimd.dma_start(out=beta_sb, in_=beta_ext.ap().to_broadcast((P, 1)))

        # ---- emission helpers ----
        def emit_tr(kb_b, g, ktts, borrow=False):
            # fp8 PE transposes write step-2 padded psum; 3 trp banks +
            # (early, while mm2 is idle) borrowed outp slots deepen the ring
            if borrow:
                trt = trp.tile([P, GP, 2, CT, P, 2], FP8, name="trt",
                               tag="trt")
            else:
                trt = outp.tile([P, GP, 2, CT, P, 2], FP8, name="trto",
                                tag="ps")
            for pr in range(GP):
                for j in range(2):
                    t = (GP * g + pr) * 2 + j
                    for dj in range(CT):
                        nc.tensor.transpose(
                            trt[:, pr, j, dj, :, 0],
                            kb_b[:, dj, t * P:(t + 1) * P],
                            ident)
            ktt = kttp.tile([P, GP, 2, CT * P], U16, name="ktt")
            cb.copy(ktt, trt.bitcast(U16), GP * 2 * CT * P, twox=True,
                    prefer=1)
            ktts[g] = ktt.bitcast(FP8)

        def emit_emm_ci(qt_b, g, ci, eng_b, ktts):
            for pr in range(GP):
                t2 = GP * g + pr
                nc.tensor.matmul(
                    eng_b[ci],
                    lhsT=qt_b[t2 // QCH][:, t2 % QCH, :,
                                         ci * P:(ci + 1) * P],
                    rhs=ktts[g][:, pr, :, ::2],
                    start=(t2 == 0),
                    stop=(t2 == NPAIR - 1),
                    perf_mode=DR,
                )

        def emit_softmax_ci(eng_b, ci):
            # softmax(max-e) = exp(min-e)/sum with beta/sum folded into the
            # fp8 attention weights; pair-transposed for the DR mm2
            mn = statp.tile([P, 1], F32)
            nc.vector.tensor_reduce(out=mn, in_=eng_b[ci],
                                    axis=mybir.AxisListType.X,
                                    op=mybir.AluOpType.min)
            ex = expp.tile([P, C], F32)
            sm = statp.tile([P, 1], F32)
            nc.scalar.activation(out=ex, in_=eng_b[ci],
                                 func=mybir.ActivationFunctionType.Exp,
                                 bias=mn, scale=-1.0, accum_out=sm)
            rc = statp.tile([P, 1], F32)
            nc.vector.reciprocal(out=rc, in_=sm)
            rb = statp.tile([P, 1], F32)
            nc.vector.tensor_mul(out=rb, in0=rc, in1=beta_sb)
            at = attp.tile([P, C], FP8)
            nc.vector.tensor_scalar_mul(out=at, in0=ex, scalar1=rb)
            atr = trp.tile([P, GP, 2, CT, P, 2], FP8, name="atr", tag="trt")
            for dj in range(CT):
                nc.tensor.transpose(atr[:, 0, 0, dj, :, 0],
                                    at[:, dj * P:(dj + 1) * P], ident)
            att = atTp.tile([P, CT, P], FP8, name="atT")
            cb.copy(att, atr[:, 0, 0, :, :, 0], CT * P)
            cb.charge(0, 800)
            cb.charge(1, 1500)
            return att

        def emit_mm2_chunk(b, kb_b, atT, deltas, ci, off, width, borrow):
            # one slice of delta = attn_scaled @ k: DR matmuls with the full
            # 256-deep contraction per 512 of width, fp8 repack.  borrow=True
            # uses a retired transpose bank as an extra (512-wide) ring slot
            if borrow:
                ps = trp.tile([P, width // OW, OW], F32, name="pst",
                              tag="trt")
            else:
                ps = outp.tile([P, width // OW, OW], F32, name="ps",
                               tag="ps")
            for q in range(width // OW):
                w0 = off + q * OW
                nc.tensor.matmul(
                    ps[:, q, :],
                    lhsT=atT[ci],
                    rhs=kb_b[:, :, w0:w0 + OW],
                    start=True, stop=True,
                    perf_mode=DR,
                )
            cb.copy(deltas[ci][:, off:off + width], ps, width)

        def store(b, deltas, ci, off, width):
            nc.sync.dma_start(
                out=out_ext[b, ci * P:(ci + 1) * P, off:off + width],
                in_=deltas[ci][:, off:off + width])

        # ---- two-batch software pipeline ----
        engsl = [engp.tile([P, CT, C], F32, name="eng", tag="eng")
                 for _ in range(BL)]
        eng = [[engsl[b][:, ci, :] for ci in range(CT)] for b in range(BL)]
        ktts = [[None] * NG for _ in range(BL)]
        deltas = [[deltap.tile([P, N], FP8, name="delta") for _ in range(CT)]
                  for _ in range(BL)]
        chunks = [(ci, w2 * DCW, DCW) for ci in range(CT)
                  for w2 in range(NDC)]

        # batch 0 energy: transposes stream behind the kb loads (ring-5 via
        # borrowed outp slots), then the per-ci energy passes + softmax
        for g in range(NG):
            emit_tr(kb[0], g, ktts[0], borrow=(g % 5 < 3))
        atT0 = []
        for ci in range(CT):
            for g in range(NG):
                emit_emm_ci(qt[0], g, ci, eng[0], ktts[0])
            atT0.append(emit_softmax_ci(eng[0], ci))

        # phase B: batch-1 transposes (trp ring only) interleaved with the
        # first NIL batch-0 mm2 chunks, in data-arrival order
        for i in range(NG):
            emit_tr(kb[1], i, ktts[1], borrow=True)
            if i < NIL:
                ci, off, width = chunks[i]
                emit_mm2_chunk(0, kb[0], atT0, deltas[0], ci, off, width,
                               False)
                if (off + width) % SCW == 0:
                    store(0, deltas[0], ci, off + width - SCW, SCW)

        # batch-1 energy per ci: ci0 finishes (and its softmax runs) while
        # the ci1 pass is still on the PE
        atT1 = []
        for ci in range(CT):
            for g in range(NG):
                emit_emm_ci(qt[1], g, ci, eng[1], ktts[1])
            atT1.append(emit_softmax_ci(eng[1], ci))

        # merged tail: remaining batch-0 chunks + all batch-1 chunks.
        # outp serves the 1024-wide chunks (ring-2), retired transpose
        # banks serve extra 512-wide chunks (ring-3)
        rest = chunks[NIL:]
        for ci in range(CT):
            for s in range(N // 1536):
                emit_mm2_chunk(1, kb[1], atT1, deltas[1], ci,
                               s * 1536, 1024, False)
                emit_mm2_chunk(1, kb[1], atT1, deltas[1], ci,
                               s * 1536 + 1024, 512, True)
                store(1, deltas[1], ci, s * 1536, 1536)
                if rest:
                    cj, off, width = rest.pop(0)
                    emit_mm2_chunk(0, kb[0], atT0, deltas[0], cj, off,
                                   width, False)
                    if (off + width) % SCW == 0:
                        store(0, deltas[0], cj, off + width - SCW, SCW)

    nc.compile()
    return nc


_NC_CACHE = None


def _get_program():
    global _NC_CACHE
    if _NC_CACHE is None:
        _NC_CACHE = _build_program()
    return _NC_CACHE


def pack_qt(Z1):
    # fp8 q^T, contraction-pair-major: qt[b, p, t2, j, c] = q[b, c, n] with
    # n = (2*t2 + j)*128 + p, matching the DoubleRow lhsT pair layout
    x = Z1.reshape(B, C, NT, P).astype(NP_FP8)
    return np.ascontiguousarray(x.transpose(0, 3, 2, 1)).reshape(
        B, P, NPAIR, 2, C)


def kernel(Z1, Zr, beta):
    Z1 = np.asarray(Z1, dtype=np.float32)
    Zr = np.asarray(Zr, dtype=np.float32)
    beta = np.asarray(beta, dtype=np.float32).reshape(1)

    qta = pack_qt(Z1)
    zrk = np.ascontiguousarray(Zr.reshape(B, C, N)).astype(NP_FP8)

    in_maps = []
    for i in range(NCORES):
        s = slice(i * BL, (i + 1) * BL)
        in_maps.append({"qt": qta[s], "zr": zrk[s], "beta": beta})

    nc = _get_program()
    res = run_bass_kernel_spmd(nc, in_maps, list(range(NCORES)))
    delta = np.concatenate(
        [np.asarray(r["out"]).astype(np.float32) for r in res.results], axis=0)
    return (Zr.reshape(B, C, N) + delta).reshape(B, C, H, W)
